# revision 1
# baseline (speedup 1.0000x reference)
"""CrossAttention Trainium2 kernel.

Full inputs -> shard (batch x head-group) over 8 NeuronCores -> Bass/Tile
kernel per core -> host-side partial-sum + bias.

Per-core shard (core c): batch b = c // 4, heads hg = (c % 4) * 4 .. +4.
Each core computes, for its 4 heads:
  QT = (x_b @ Wq[:, cols])^T          [256, 2048]  (f32r)
  KT = (ctx_b @ Wk[:, cols])^T        [256, 2048]  (f32r)
  V' = [ctx_b @ Wv[:, cols] | 1]      [2048, 4, 65] (fp16, ones col -> denom)
  per head: S^T = K_h Q_h^T (f32r), P = exp(S^T * scale) (fp16),
            O'^T = V'^T P^T  (PSUM fp32; row 64 = softmax denominator)
  normalize O by 1/denom (broadcast via tiny rank-2 matmul), then
  partial_out = O_norm^T.T @ Wout[rows] -> [2048, 1024]  (f32r matmuls)
Host: out[b] = sum of the 4 partials + bias.
"""

import numpy as np

import concourse.bass as bass
import concourse.bacc as bacc
import concourse.mybir as mybir
import concourse.tile as tile
from concourse import bass_utils

B, S, D = 2, 2048, 1024
H_TOT, DH = 16, 64
N_CORES = 8
H = 4                 # heads per core
I = H * DH            # 256: inner dim per core
SCALE = float(DH) ** -0.5
KD = D // 128         # 8 contraction tiles over model dim
NQ = S // 512         # 4 query chunks of 512
NKV = S // 128        # 16 kv tiles of 128

F32 = mybir.dt.float32
F32R = mybir.dt.float32r
F16 = mybir.dt.float16

AF = mybir.ActivationFunctionType

_CACHE = {}


def _emit(nc, tc, xT, cT, wq, wk, wv, wo, out):
    pp = tc.alloc_tile_pool(name="persist", bufs=1)

    # ---- persistent SBUF ----
    qt = [pp.tile([128, S], F32R, tag=f"qt{m}", name=f"qt{m}") for m in range(2)]
    kt = [pp.tile([128, S], F32R, tag=f"kt{m}", name=f"kt{m}") for m in range(2)]
    vp = pp.tile([128, NKV, H, 65], F16, tag="vp")
    # per-head softmax denominators: head h's row lives at partition 32h
    # (32-aligned bases are required), so ln/exp run 4-rows-wide instead of
    # on a single partition.  memset 1.0 keeps the junk rows finite.
    # fp16: D ~ 3e3 << 65504, and 5e-4 rounding on the denominator is fine.
    d4 = pp.tile([128, S], F16, tag="d4")
    nc.vector.memset(d4[:], 1.0)
    ones_t = pp.tile([128, 128], F16, tag="ones_t")
    nc.vector.memset(ones_t[:], 1.0)
    # unnormalized O^T halves (fp16: feeds the all-fp16 out-projection)
    o_un = [pp.tile([128, S], F16, tag=f"o_un{m}", name=f"o_un{m}") for m in range(2)]

    # ones column of V' (softmax denominator trick)
    nc.vector.memset(vp[:, :, :, 64], 1.0)

    NP = 8                 # P chunk tiles per head
    PER = NKV // NP        # kv tiles per P chunk
    p2 = tc.alloc_tile_pool(name="ph2", bufs=1)
    ps_sc = None  # created after the QT phase frees its 8 PSUM banks
    wo_s = p2.tile([128, 2, D], F16, tag="wo")

    def score_tile(h, Ph, i):
        # S^T tile i = K_h Q_h^T, exp'd into fp16 P chunk while evicting
        mt, po = h // 2, (h % 2) * 64
        for g in range(2):
            sg = ps_sc.tile([128, 1024], F32, tag="sc", name="sg")
            for sub in range(2):
                q0 = g * 1024 + sub * 512
                nc.tensor.matmul(
                    sg[:, sub * 512:(sub + 1) * 512],
                    kt[mt][po:po + 64, i * 128:(i + 1) * 128],
                    qt[mt][po:po + 64, q0:q0 + 512],
                    start=True, stop=True,
                )
            nc.scalar.activation(
                Ph[i // PER][:, i % PER, g * 1024:(g + 1) * 1024], sg[:],
                AF.Exp, scale=SCALE,
            )

    def pv_tile(h, Ph, o_ps, i):
        # O'^T += V'_i^T P_i^T ; V' stationary across the 4 q-chunks.
        for qc in range(NQ):
            nc.tensor.matmul(
                o_ps[:, qc * 512:(qc + 1) * 512],
                vp[:, i, h, :],
                Ph[i // PER][:, i % PER, qc * 512:(qc + 1) * 512],
                start=(i == 0), stop=(i == NKV - 1),
            )

    def emit_scores(h, Ph, hook=None):
        for i in range(NKV):
            if hook is not None:
                hook(i)
            score_tile(h, Ph, i)

    def emit_pv(h, Ph, ps_o):
        mt, po = h // 2, (h % 2) * 64
        o_ps = ps_o.tile([65, S], F32, tag="o", name=f"ops{h}")
        for i in range(NKV):
            pv_tile(h, Ph, o_ps, i)
        # evictions: DVE mid-stream; ACT for the last head (its queue is
        # empty after the final exp, while DVE is busy with out0 copies -
        # this keeps norm(1) off the DVE queue's critical path)
        if h == H - 1:
            # d first: ln/exp/r chain starts while o_un still evicting
            nc.scalar.copy(d4[32 * h:32 * h + 1, :], o_ps[64:65, :])
            nc.scalar.copy(o_un[mt][po:po + 64, :], o_ps[0:64, :])
        else:
            nc.vector.tensor_copy(o_un[mt][po:po + 64, :], o_ps[0:64, :])
            nc.vector.tensor_copy(d4[32 * h:32 * h + 1, :], o_ps[64:65, :])

    def emit_outproj(osp, out, ps_op):
        # partial_out[q, d] = o_un^T @ wo (both pairs accumulated in PSUM;
        # fp16 partials summed on the host across cores)
        for qi in range(S // 128):
            for n in range(2):
                op = ps_op.tile([128, 512], F32, tag="sc", name="op")
                for m in range(2):
                    nc.tensor.matmul(
                        op[:],
                        o_un[m][:, qi * 128:(qi + 1) * 128],
                        wo_s[:, m, n * 512:(n + 1) * 512],
                        start=(m == 0), stop=(m == 1),
                    )
                ost = osp.tile([128, 512], F16, tag="ost", name="ost")
                if (qi * 2 + n) % 2 == 1:
                    nc.scalar.copy(ost[:], op[:])
                else:
                    nc.vector.tensor_copy(ost[:], op[:])
                nc.sync.dma_start(
                    out[qi * 128:(qi + 1) * 128, n * 512:(n + 1) * 512],
                    ost[:])

    def emit_norm_pair(mt):
        # r = 1/D = exp(-ln(D)) for heads 2mt, 2mt+1 (adjacent d_cat
        # slices -> batched ACT calls).  Rank-1 matmul (ones ⊗ r)
        # broadcasts r over all 128 partitions; each head's 64-row half of
        # o_un is scaled in place.  Called mid-way through a later head's
        # score loop so the ACT-queue stall behind PV is hidden.
        dp_sl = d4[64 * mt:64 * mt + 33, :]
        nc.scalar.activation(dp_sl, dp_sl, AF.Ln)   # in-place ln(D)
        nc.scalar.activation(dp_sl, dp_sl, AF.Exp, scale=-1.0)  # now r=1/D
        for g in range(2):
            for hh in (2 * mt, 2 * mt + 1):
                po = (hh % 2) * 64
                rp = 32 * hh
                R_ps = ps_sc.tile([128, 1024], F32, tag="sc", name="R_ps")
                for sub in range(2):
                    q0 = g * 1024 + sub * 512
                    nc.tensor.matmul(
                        R_ps[:, sub * 512:(sub + 1) * 512],
                        ones_t[rp:rp + 1, :],
                        d4[rp:rp + 1, q0:q0 + 512],
                        start=True, stop=True,
                        tile_position=(rp, 0),
                    )
                nc.vector.tensor_mul(
                    o_un[mt][po:po + 64, g * 1024:(g + 1) * 1024],
                    o_un[mt][po:po + 64, g * 1024:(g + 1) * 1024],
                    R_ps[po:po + 64, :],
                )

    def new_P(h):
        # chunked P tiles: head h+1's scores into chunk c wait only for
        # head h's PV to finish reading chunk c.  First 3 tags are
        # double-buffered so the next head's scores lead PV by ~3 chunks.
        return [p2.tile([128, PER, S], F16, tag=f"P{c}", name=f"P{h}_{c}",
                        bufs=(2 if c < 3 else 1))
                for c in range(NP)]

    P0 = new_P(0)

    # ================= Phase 1: projections =================
    with tc.tile_pool(name="ph1", bufs=1) as p1:
        wq_s = p1.tile([128, KD, I], F16, tag="wq")
        wk_s = p1.tile([128, KD, I], F16, tag="wk")
        wv_s = p1.tile([128, KD, I], F16, tag="wv")
        nc.sync.dma_start(wq_s[:], wq.rearrange("(k p) i -> p k i", p=128))
        ct_s = p1.tile([128, KD, S], F16, tag="ct")

        # -- QT half m: xT streamed K-outer into 4 psum accumulators
        ps_sc = tc.alloc_tile_pool(name="ps_sc", bufs=2, space="PSUM")
        with (
            tc.tile_pool(name="xs", bufs=4) as xsp,
            tc.tile_pool(name="ps1", bufs=1, space="PSUM") as ps1,
        ):
            def qt_half(m):
                ps_q = [ps1.tile([128, 512], F32, tag=f"pk{j}", name=f"psq{j}")
                        for j in range(NQ)]
                for k in range(KD):
                    xt_k = xsp.tile([128, S], F16, tag="xt", name="xt_k")
                    nc.sync.dma_start(xt_k[:], xT[k * 128:(k + 1) * 128, :])
                    if m == 0:
                        nc.sync.dma_start(ct_s[:, k], cT[k * 128:(k + 1) * 128, :])
                    if m == 0 and k == 0:
                        nc.sync.dma_start(
                            wk_s[:], wk.rearrange("(k p) i -> p k i", p=128))
                        nc.sync.dma_start(
                            wv_s[:], wv.rearrange("(k p) i -> p k i", p=128))
                    for qc in range(NQ):
                        nc.tensor.matmul(
                            ps_q[qc][:],
                            wq_s[:, k, m * 128:(m + 1) * 128],
                            xt_k[:, qc * 512:(qc + 1) * 512],
                            start=(k == 0), stop=(k == KD - 1),
                        )
                for qc in range(NQ):
                    nc.vector.tensor_copy(
                        qt[m][:, qc * 512:(qc + 1) * 512], ps_q[qc][:])

            def kt_chunk(m, qc):
                ps_k = ps1.tile([128, 512], F32, tag=f"pk{qc}", name="psk")
                for k in range(KD):
                    nc.tensor.matmul(
                        ps_k[:],
                        wk_s[:, k, m * 128:(m + 1) * 128],
                        ct_s[:, k, qc * 512:(qc + 1) * 512],
                        start=(k == 0), stop=(k == KD - 1),
                    )
                nc.vector.tensor_copy(
                    kt[m][:, qc * 512:(qc + 1) * 512], ps_k[:])

            def v_group(mv):
                # V natural [kv, 256] -> fp16 V' with per-head stride 65
                ps_v = ps1.tile([128, 256], F32, tag=f"pk{mv % 4}", name="psv")
                for k in range(KD):
                    nc.tensor.matmul(
                        ps_v[:],
                        ct_s[:, k, mv * 128:(mv + 1) * 128],
                        wv_s[:, k, :],
                        start=(k == 0), stop=(k == KD - 1),
                    )
                nc.vector.tensor_copy(
                    vp[:, mv, :, 0:64],
                    ps_v.rearrange("p (h d) -> p h d", h=H),
                )

            qt_half(0)
            # interleave KT-m0 chunks with head-0 score tiles: only ~10 MMs
            # sit ahead of the first exp, so ACT starts as soon as the input
            # DMA front finishes.  V-proj and the m=1 projections fill PE
            # slack under head-0's exp window.
            for qc in range(NQ):
                kt_chunk(0, qc)
                for i in range(4 * qc, 4 * qc + 4):
                    score_tile(0, P0, i)
            for mv in range(NKV):
                v_group(mv)
            qt_half(1)
            for qc in range(NQ):
                kt_chunk(1, qc)

    # ================= Phase 2: attention =================
    with (
        tc.tile_pool(name="ps_o", bufs=1, space="PSUM") as ps_o,
        tc.tile_pool(name="ostage", bufs=4) as osp,
    ):
        nc.sync.dma_start(wo_s[:], wo.rearrange("(k p) d -> p k d", p=128))
        # cross-head pipeline: scores_h outranks pv_{h-1} in priority, so
        # the scheduler interleaves them chunk-by-chunk via P-slot readiness
        # and ACT never starves at head boundaries.

        def pair0_hook(i):
            if i == 8:
                emit_norm_pair(0)

        Pt = {0: P0}
        for h in range(1, H):
            Pt[h] = new_P(h)
            hook = pair0_hook if h == 3 else None
            emit_scores(h, Pt[h], hook=hook)
            emit_pv(h - 1, Pt[h - 1], ps_o)
        emit_pv(H - 1, Pt[H - 1], ps_o)
        emit_norm_pair(1)
        emit_outproj(osp, out, ps_sc)

    ps_sc.release()
    p2.release()
    pp.release()


def build_program():
    nc = bacc.Bacc("TRN2", target_bir_lowering=False, debug=False,
                   num_devices=N_CORES)
    xT = nc.dram_tensor("xT", [D, S], F16, kind="ExternalInput").ap()
    cT = nc.dram_tensor("cT", [D, S], F16, kind="ExternalInput").ap()
    wq = nc.dram_tensor("wq", [D, I], F16, kind="ExternalInput").ap()
    wk = nc.dram_tensor("wk", [D, I], F16, kind="ExternalInput").ap()
    wv = nc.dram_tensor("wv", [D, I], F16, kind="ExternalInput").ap()
    wo = nc.dram_tensor("wo", [I, D], F16, kind="ExternalInput").ap()
    out = nc.dram_tensor("out", [S, D], F16, kind="ExternalOutput").ap()
    with tile.TileContext(nc) as tc:
        _emit(nc, tc, xT, cT, wq, wk, wv, wo, out)
    nc.compile()
    return nc


def make_in_maps(x, context, Wq, Wkv, Wout):
    x = np.asarray(x, dtype=np.float32)
    context = np.asarray(context, dtype=np.float32)
    Wq = np.asarray(Wq, dtype=np.float32)
    Wkv = np.asarray(Wkv, dtype=np.float32)
    Wout = np.asarray(Wout, dtype=np.float32)
    in_maps = []
    for c in range(N_CORES):
        b, hg = c // 4, (c % 4) * H
        cols = slice(hg * DH, (hg + H) * DH)
        in_maps.append({
            "xT": np.ascontiguousarray(x[b].T).astype(np.float16),
            "cT": np.ascontiguousarray(context[b].T).astype(np.float16),
            "wq": np.ascontiguousarray(Wq[:, cols]).astype(np.float16),
            "wk": np.ascontiguousarray(Wkv[:, cols]).astype(np.float16),
            "wv": np.ascontiguousarray(Wkv[:, D + cols.start:D + cols.stop]).astype(np.float16),
            "wo": np.ascontiguousarray(Wout[cols, :]).astype(np.float16),
        })
    return in_maps


def kernel(x, context, Wq, Wkv, Wout, bout):
    if "nc" not in _CACHE:
        _CACHE["nc"] = build_program()
    nc = _CACHE["nc"]
    in_maps = make_in_maps(x, context, Wq, Wkv, Wout)
    res = None
    for attempt in range(4):
        try:
            res = bass_utils.run_bass_kernel_spmd(
                nc, in_maps, core_ids=list(range(N_CORES)))
            break
        except Exception:
            # transient NRT/axon device errors occasionally wedge a dispatch;
            # reset the PJRT client and retry
            if attempt == 3:
                raise
            import time
            time.sleep(20.0)
            try:
                import jax
                import jax.extend.backend as _jeb
                jax.clear_caches()
                _jeb.clear_backends()
            except Exception:
                pass
    bout = np.asarray(bout, dtype=np.float32)
    out = np.empty((B, S, D), dtype=np.float32)
    for b in range(B):
        acc = res.results[4 * b]["out"].astype(np.float32)
        for c in range(4 * b + 1, 4 * b + 4):
            acc += res.results[c]["out"]
        out[b] = acc + bout[None, :]
    return out

